# revision 10
# baseline (speedup 1.0000x reference)
"""GBST layer (pooling) Trainium2 Bass/Tile kernel.

Math (per sample, x [512, 8192]):
  y = conv1d(x, W[512,512,5], b, VALID)                    # [512, 8188]
  r[l] = score . y[:, l]                                   # conv'd scores
  For w in {1,2,3}: cand_w = block-mean(y, w); s_w = block-mean(r, w)
  att = softmax over the 3 per-position scores; out[l] = sum_w att_w[l] * cand_w[bw(l)]
  out_ds = pairwise mean of out                            # [512, 4096]

Strategy: 1 sample per NeuronCore (8 cores, batch-parallel).
  - conv as 5 shifted bf16 matmuls per (oc, ic) chunk pair on PE (1280 MMs)
  - y kept fully resident in SBUF as bf16 [4][128, 8208] (zero-padded tail
    reproduces the reference's zero-pad semantics exactly)
  - r via PE (score^T @ y) into a persistent SBUF row, re-read in an "L6"
    layout [128, 66] where partition j holds positions 768c+6j+u — every
    pooling width (2, 3) is then an intra-partition strided op
  - softmax + downsample folded into 4 coefficient rows A,B,D,E:
      out_ds[:,p] = A[p]*y[:,2p] + B[p]*y[:,2p+1] + D[p]*S3[:,g0(p)] + E[p]*S3[:,g1(p)]
    with S3 = stride-3 running sums of y, g0=floor(2p/3), g1=floor((2p+1)/3)
  - coefficient rows reordered to natural p-order straight into DRAM with a
    single strided DMA, then broadcast to 128 partitions with stride-0 reads

This walrus build caps semaphore waits per instruction very low, so
_fix_wait_overflow() hoists excess waits onto injected same-engine NOPs
placed immediately before the overflowing instruction (safe: no intervening
same-engine instructions, so producers can't depend on anything between).
"""

import numpy as np
import ml_dtypes

import concourse.bass as bass
import concourse.mybir as mybir
from concourse.tile import TileContext

BF16 = mybir.dt.bfloat16
F32 = mybir.dt.float32
AF = mybir.ActivationFunctionType
ALU = mybir.AluOpType

N_CORES = 8
E, L, KS = 512, 8192, 5
LC = L - KS + 1          # 8188 valid conv outputs
LPAD = 8208              # y cols incl. zero tail (S3 reads up to col 8204)
NCB = 11                 # L6-layout column blocks: 768*11 = 8448 >= L
RPAD = 768 * NCB         # padded r length
PCO = 384 * NCB          # coeff row length (>= 4096)
NT = 16                  # conv tiles of 512 positions
OUTL = L // 2            # 4096
NTP = 8                  # combine tiles of 512 output cols
NS3T = 345               # S3 cols materialized per combine tile

_BUILT = None


def _sap(tile_ap, col_off, dims):
    """Strided SBUF AP on a pool tile: partition dim + custom free dims."""
    pitch, nparts = tile_ap.ap[0]
    return bass.AP(tile_ap.tensor, tile_ap.offset + col_off, [[pitch, nparts]] + dims)


_LIM1 = {"InstNoOp", "InstDrain", "InstEventSemaphore"}


def _fix_wait_overflow(nc):
    """Split >limit semaphore waits onto injected same-engine NOPs."""
    cnt = 0
    for f in nc.m.functions:
        for b in f.blocks:
            newlist = []
            for inst in b.instructions:
                si = inst.sync_info
                if si is not None and si.on_wait:
                    lim = 1
                    waits = list(si.on_wait)
                    while len(waits) > lim:
                        w = waits.pop(0)
                        nop = mybir.InstNoOp(name=f"wfx-{cnt}")
                        cnt += 1
                        nop.engine = inst.engine
                        nop.sync_info = mybir.SyncInfo(on_wait=[w], on_update=[])
                        newlist.append(nop)
                    if cnt and len(waits) != len(si.on_wait):
                        inst.sync_info = mybir.SyncInfo(
                            on_wait=waits, on_update=list(si.on_update)
                        )
                newlist.append(inst)
            b.instructions[:] = newlist
    return cnt


def _build_bass(fix_waits=True):
    nc = bass.Bass("TRN2", target_bir_lowering=False, num_devices=N_CORES)

    xb = nc.dram_tensor("xb", [E, L], BF16, kind="ExternalInput")
    wsb = nc.dram_tensor("wsb", [128, KS * 4 * 4 * 128], BF16, kind="ExternalInput")
    scs = nc.dram_tensor("scs", [128, 4], BF16, kind="ExternalInput")
    bis = nc.dram_tensor("bis", [128, 4], F32, kind="ExternalInput")
    out_d = nc.dram_tensor("out", [E, OUTL], F32, kind="ExternalOutput")
    coef_d = nc.dram_tensor("coef", [4, PCO], F32)
    r_d = nc.dram_tensor("r_scratch", [RPAD], F32)

    with TileContext(nc) as tc:
        with (
            tc.tile_pool(name="const", bufs=1) as kpool,
            tc.tile_pool(name="ybuf", bufs=1) as ypool,
            tc.tile_pool(name="xin", bufs=3) as xpool,
            tc.tile_pool(name="ps", bufs=3, space="PSUM") as pspool,
            tc.tile_pool(name="psr", bufs=2, space="PSUM") as prpool,
            tc.tile_pool(name="sc", bufs=1) as spool,
            tc.tile_pool(name="cf", bufs=2) as cpool,
            tc.tile_pool(name="s3", bufs=2) as s3pool,
            tc.tile_pool(name="ot", bufs=3) as opool,
        ):
            w_sb = kpool.tile([128, KS * 4 * 4 * 128], BF16, tag="w")
            nc.sync.dma_start(out=w_sb[:], in_=wsb[:])
            sc_sb = kpool.tile([128, 4], BF16, tag="sc")
            nc.sync.dma_start(out=sc_sb[:], in_=scs[:])
            bi_sb = kpool.tile([128, 4], F32, tag="bi")
            nc.sync.dma_start(out=bi_sb[:], in_=bis[:])

            ys = [
                ypool.tile([128, LPAD], BF16, name=f"y{c}", tag=f"y{c}")
                for c in range(4)
            ]
            for c in range(4):
                nc.gpsimd.memset(ys[c][:, LC:LPAD], 0.0)
            zr = spool.tile([1, RPAD - LC], F32, tag="zr")
            nc.gpsimd.memset(zr[:], 0.0)
            nc.sync.dma_start(out=bass.AP(r_d, LC, [[1, RPAD - LC]]), in_=zr[:1, :])

            # ---- phase 1: conv + scores ----
            def emit_r(t):
                n0 = 512 * t
                n = min(512, LC - n0)
                pr = prpool.tile([1, 512], F32, tag="pr")
                for cc in range(4):
                    nc.tensor.matmul(
                        pr[:, :n],
                        lhsT=sc_sb[:, cc : cc + 1],
                        rhs=ys[cc][:, n0 : n0 + n],
                        start=(cc == 0),
                        stop=(cc == 3),
                    )
                rsb = xpool.tile([1, 512], F32, tag="rsb")
                nc.scalar.activation(rsb[:1, :n], pr[:1, :n], AF.Copy)
                nc.sync.dma_start(out=bass.AP(r_d, n0, [[1, n]]), in_=rsb[:1, :n])

            for t in range(NT):
                n0 = 512 * t
                n = min(512, LC - n0)
                xw = min(516, L - n0)
                xt = xpool.tile([128, 4 * 516], BF16, tag="xt")
                for ic in range(4):
                    nc.sync.dma_start(
                        out=xt[:, ic * 516 : ic * 516 + xw],
                        in_=xb[128 * ic : 128 * (ic + 1), n0 : n0 + xw],
                    )
                for oc in range(4):
                    py = pspool.tile([128, 512], F32, tag="py")
                    first = True
                    for ic in range(4):
                        for k in range(KS):
                            nc.tensor.matmul(
                                py[:, :n],
                                lhsT=w_sb[
                                    :,
                                    ((k * 4 + ic) * 4 + oc) * 128 : ((k * 4 + ic) * 4 + oc + 1) * 128,
                                ],
                                rhs=xt[:, ic * 516 + k : ic * 516 + k + n],
                                start=first,
                                stop=(ic == 3 and k == KS - 1),
                            )
                            first = False
                    nc.scalar.activation(
                        ys[oc][:, n0 : n0 + n], py[:, :n], AF.Identity,
                        bias=bi_sb[:, oc : oc + 1], scale=1.0,
                    )
                if t >= 1:
                    emit_r(t - 1)
            emit_r(NT - 1)

            # ---- phase 2: score math in L6 layout ----
            # r6[j, 6c+u] = r[768c + 6j + u]
            r6 = spool.tile([128, 66], F32, tag="r6")
            for u in range(6):
                nc.sync.dma_start(
                    out=_sap(r6, u, [[6, NCB]]),
                    in_=bass.AP(r_d, u, [[6, 128], [768, NCB]]),
                )
            e1 = spool.tile([128, 66], F32, tag="e1")
            nc.scalar.activation(e1[:], r6[:], AF.Exp)
            # s2h[j,3c+v] = r6[,6c+2v] + r6[,6c+2v+1];  e2 = exp(s2h/2)
            s2h = spool.tile([128, 33], F32, tag="s2h")
            nc.vector.tensor_add(
                out=s2h[:],
                in0=_sap(r6, 0, [[6, NCB], [2, 3]]),
                in1=_sap(r6, 1, [[6, NCB], [2, 3]]),
            )
            e2 = spool.tile([128, 33], F32, tag="e2")
            nc.scalar.activation(e2[:], s2h[:], AF.Exp, scale=0.5)
            # s3h[j,2c+w] = sum of r6[,6c+3w+{0,1,2}];  e3 = exp(s3h/3)
            s3h = spool.tile([128, 22], F32, tag="s3h")
            nc.vector.tensor_add(
                out=s3h[:],
                in0=_sap(r6, 0, [[6, NCB], [3, 2]]),
                in1=_sap(r6, 1, [[6, NCB], [3, 2]]),
            )
            nc.vector.tensor_add(
                out=s3h[:], in0=s3h[:], in1=_sap(r6, 2, [[6, NCB], [3, 2]])
            )
            e3 = spool.tile([128, 22], F32, tag="e3")
            nc.scalar.activation(e3[:], s3h[:], AF.Exp, scale=1.0 / 3.0)

            # den = e1 + expand2(e2) + expand3(e3), then rec = 1/den
            den = spool.tile([128, 66], F32, tag="den")
            for v in range(3):
                nc.vector.tensor_add(
                    out=_sap(den, 2 * v, [[6, NCB], [1, 2]]),
                    in0=_sap(e1, 2 * v, [[6, NCB], [1, 2]]),
                    in1=_sap(e2, v, [[3, NCB], [0, 2]]),
                )
            for w in range(2):
                nc.vector.tensor_add(
                    out=_sap(den, 3 * w, [[6, NCB], [1, 3]]),
                    in0=_sap(den, 3 * w, [[6, NCB], [1, 3]]),
                    in1=_sap(e3, w, [[2, NCB], [0, 3]]),
                )
            rec = spool.tile([128, 66], F32, tag="rec")
            nc.vector.reciprocal(rec[:], den[:])

            # t1 = e1*rec ; recsum[3c+v] = rec[6c+2v]+rec[6c+2v+1]
            t1 = spool.tile([128, 66], F32, tag="t1")
            nc.vector.tensor_mul(out=t1[:], in0=e1[:], in1=rec[:])
            recsum = spool.tile([128, 33], F32, tag="recsum")
            nc.vector.tensor_add(
                out=recsum[:],
                in0=_sap(rec, 0, [[6, NCB], [2, 3]]),
                in1=_sap(rec, 1, [[6, NCB], [2, 3]]),
            )
            # e2r = 0.25 * e2 * recsum
            e2r = spool.tile([128, 33], F32, tag="e2r")
            nc.vector.scalar_tensor_tensor(
                out=e2r[:], in0=e2[:], scalar=0.25, in1=recsum[:],
                op0=ALU.mult, op1=ALU.mult,
            )
            # A = 0.5*t1[even] + e2r ; B = 0.5*t1[odd] + e2r
            Ac = spool.tile([128, 33], F32, tag="Ac")
            Bc = spool.tile([128, 33], F32, tag="Bc")
            nc.vector.scalar_tensor_tensor(
                out=Ac[:], in0=_sap(t1, 0, [[6, NCB], [2, 3]]),
                scalar=0.5, in1=e2r[:], op0=ALU.mult, op1=ALU.add,
            )
            nc.vector.scalar_tensor_tensor(
                out=Bc[:], in0=_sap(t1, 1, [[6, NCB], [2, 3]]),
                scalar=0.5, in1=e2r[:], op0=ALU.mult, op1=ALU.add,
            )
            # D[3c+v] = (1/6) e3[2c + (0,0,1)v] * rec[6c+2v]
            # E[3c+v] = (1/6) e3[2c + (0,1,1)v] * rec[6c+2v+1]
            Dc = spool.tile([128, 33], F32, tag="Dc")
            Ec = spool.tile([128, 33], F32, tag="Ec")
            for v, (w0, w1) in enumerate([(0, 0), (0, 1), (1, 1)]):
                nc.vector.scalar_tensor_tensor(
                    out=_sap(Dc, v, [[3, NCB]]),
                    in0=_sap(e3, w0, [[2, NCB]]),
                    scalar=1.0 / 6.0,
                    in1=_sap(rec, 2 * v, [[6, NCB]]),
                    op0=ALU.mult, op1=ALU.mult,
                )
                nc.vector.scalar_tensor_tensor(
                    out=_sap(Ec, v, [[3, NCB]]),
                    in0=_sap(e3, w1, [[2, NCB]]),
                    scalar=1.0 / 6.0,
                    in1=_sap(rec, 2 * v + 1, [[6, NCB]]),
                    op0=ALU.mult, op1=ALU.mult,
                )

            # ---- reorder coeff tiles to natural p-order in DRAM ----
            # p = 384c + 3j + v  <-  tile[j, 3c+v]; write iterates (j, c, v)
            for i, tcf in enumerate([Ac, Bc, Dc, Ec]):
                nc.sync.dma_start(
                    out=bass.AP(coef_d, i * PCO, [[3, 128], [384, NCB], [1, 3]]),
                    in_=tcf[:],
                )

            # ---- phase 3: combine ----
            for tp in range(NTP):
                p0 = 512 * tp
                ms = (1024 * tp) // 3  # S3 base block for this tile
                cofs = []
                for i, nm in enumerate(["Ab", "Bb", "Db", "Eb"]):
                    cb = cpool.tile([128, 512], F32, tag=nm)
                    nc.sync.dma_start(
                        out=cb[:],
                        in_=bass.AP(coef_d, i * PCO + p0, [[0, 128], [1, 512]]),
                    )
                    cofs.append(cb)
                Ab, Bb, Db, Eb = cofs
                for cc in range(4):
                    yt = ys[cc]
                    # S3 tile: stride-3 sums of y for blocks [ms, ms+NS3T)
                    s3t = s3pool.tile([128, NS3T], F32, tag="s3t")
                    nc.gpsimd.tensor_add(
                        out=s3t[:],
                        in0=_sap(yt, 3 * ms, [[3, NS3T]]),
                        in1=_sap(yt, 3 * ms + 1, [[3, NS3T]]),
                    )
                    nc.gpsimd.tensor_add(
                        out=s3t[:], in0=s3t[:], in1=_sap(yt, 3 * ms + 2, [[3, NS3T]])
                    )
                    ot = opool.tile([128, 512], F32, tag="ot")
                    tb = opool.tile([128, 512], F32, tag="tb")
                    nc.vector.tensor_mul(
                        out=ot[:], in0=_sap(yt, 2 * p0, [[2, 512]]), in1=Ab[:]
                    )
                    nc.vector.tensor_mul(
                        out=tb[:], in0=_sap(yt, 2 * p0 + 1, [[2, 512]]), in1=Bb[:]
                    )
                    nc.vector.tensor_add(out=ot[:], in0=ot[:], in1=tb[:])
                    for v in range(3):
                        nq = len(range(v, 512, 3))
                        g0 = (1024 * tp + 2 * v) // 3 - ms
                        g1 = (1024 * tp + 2 * v + 1) // 3 - ms
                        td = opool.tile([128, 171], F32, tag="td")
                        te = opool.tile([128, 171], F32, tag="te")
                        nc.vector.scalar_tensor_tensor(
                            out=td[:, :nq],
                            in0=_sap(s3t, g0, [[2, nq]]),
                            scalar=0.0,
                            in1=_sap(Db, v, [[3, nq]]),
                            op0=ALU.add, op1=ALU.mult,
                        )
                        nc.vector.scalar_tensor_tensor(
                            out=te[:, :nq],
                            in0=_sap(s3t, g1, [[2, nq]]),
                            scalar=0.0,
                            in1=_sap(Eb, v, [[3, nq]]),
                            op0=ALU.add, op1=ALU.mult,
                        )
                        nc.vector.tensor_add(
                            out=td[:, :nq], in0=td[:, :nq], in1=te[:, :nq]
                        )
                        nc.vector.tensor_add(
                            out=_sap(ot, v, [[3, nq]]),
                            in0=_sap(ot, v, [[3, nq]]),
                            in1=td[:, :nq],
                        )
                    nc.sync.dma_start(
                        out=out_d[128 * cc : 128 * (cc + 1), p0 : p0 + 512], in_=ot[:]
                    )

    if fix_waits:
        _fix_wait_overflow(nc)
    return nc


def _prep_inputs(x, conv_w, conv_b, score_w):
    """Per-core input maps. Core b processes sample b."""
    bf = ml_dtypes.bfloat16
    wT = np.ascontiguousarray(conv_w.transpose(1, 0, 2))  # [in, out, k]
    wsb = np.empty((128, KS * 4 * 4 * 128), dtype=bf)
    for k in range(KS):
        for ic in range(4):
            for oc in range(4):
                off = ((k * 4 + ic) * 4 + oc) * 128
                wsb[:, off : off + 128] = wT[
                    128 * ic : 128 * (ic + 1), 128 * oc : 128 * (oc + 1), k
                ].astype(bf)
    scs = np.ascontiguousarray(score_w.reshape(4, 128).T).astype(bf)
    bis = np.ascontiguousarray(conv_b.reshape(4, 128).T.astype(np.float32))
    maps = []
    for b in range(N_CORES):
        maps.append({"xb": x[b].astype(bf), "wsb": wsb, "scs": scs, "bis": bis})
    return maps


def kernel(x, conv_w, conv_b, score_w):
    global _BUILT
    from concourse.bass_utils import run_bass_kernel_spmd

    if _BUILT is None:
        _BUILT = _build_bass()
    nc = _BUILT
    x = np.asarray(x, dtype=np.float32)
    maps = _prep_inputs(
        x,
        np.asarray(conv_w, dtype=np.float32),
        np.asarray(conv_b, dtype=np.float32),
        np.asarray(score_w, dtype=np.float32),
    )
    res = run_bass_kernel_spmd(nc, maps, core_ids=list(range(N_CORES)))
    out = np.stack([r["out"] for r in res.results], axis=0)
    return out.astype(np.float32)
